# revision 4
# baseline (speedup 1.0000x reference)
"""Trainium2 Bass kernel for nn_LocalAttention (block-local attention, P=7).

Strategy
--------
Data-parallel over batch b: 16 batches -> 8 cores x 2 batches.

Host side: fmap is permuted to X^T layout (b, c, cols) with cols ordered
(t, v, p) so each attention block (t,v) is 7 contiguous columns. Weights are
pre-transposed (contraction dim c / inner on partitions); the attention scale
DH**-0.5 = 0.125 is folded into Wq. The softmax mask (+32 on block-diagonal,
-32 uniform) is passed as two rank-15 factor matrices so the PE adds it into
the score PSUM with one extra accumulating matmul; exp() then yields ~1e-14
off block-diagonal, i.e. exact-enough masking with zero vector-engine work.

On chip per 490-column chunk (5 groups of 14 blocks = 98 cols):
  Q^T/K^T (inner x cols) and V (cols x inner) projections in fp32r,
  S^T = K^T.T @ Q^T per (group, head) in bf16 accumulated onto the mask
  matmul, one batched Exp per PSUM bank (ScalarE, psum->sbuf bf16),
  denominators via ones-matmul broadcast to 64 partitions (head-pair packed),
  AV matmuls head-pair packed, VectorE reciprocal + tensor_mul (= fused
  softmax-normalize + PSUM evacuation), fp32r output projection + bias,
  DMA out.  Output is un-permuted on host.
"""

import os
import sys

for _p in ("/opt/trn_rl_repo", "/root/.axon_site/_ro/trn_rl_repo"):
    if os.path.isdir(_p) and _p not in sys.path:
        sys.path.insert(0, _p)

import numpy as np

import concourse.bacc as bacc
import concourse.mybir as mybir
import concourse.tile as tile
from concourse import bass_utils

F32 = mybir.dt.float32
F32R = mybir.dt.float32r
BF16 = mybir.dt.bfloat16

# problem constants
B, C, T, V = 16, 256, 196, 25
P, H, DH = 7, 8, 64
INNER = H * DH  # 512
NCORES = 8
BPC = B // NCORES  # batches per core = 2
COLS = T * V  # 4900 columns per batch (t, v, p)-ordered
CHUNK = 490  # columns per processing chunk
NCHUNK = COLS // CHUNK  # 10
GCOL = 98  # columns per attention group (14 blocks of 7)
NG = CHUNK // GCOL  # 5 groups per chunk
NBLK = GCOL // P  # 14 blocks per group
MASKC = 32.0  # block-diagonal boost; exp bias is -MASKC via uniform rank-1 term

_CACHE: dict = {}


def _build_program(repeat: int = 1, ablate: frozenset = frozenset(), psm_bufs: int = 4, pss_bufs: int = 3, psd_bufs: int = 1, evac: str = 'alt', sbuf_bufs: int = 2, hwloop: bool = True):
    nc = bacc.Bacc("TRN2", target_bir_lowering=False, debug=False)

    xT = nc.dram_tensor("xT", [BPC, C, COLS], F32R, kind="ExternalInput")
    wqT = nc.dram_tensor("wqT", [C, INNER], F32R, kind="ExternalInput")
    wkT = nc.dram_tensor("wkT", [C, INNER], F32R, kind="ExternalInput")
    wvT = nc.dram_tensor("wvT", [C, INNER], F32R, kind="ExternalInput")
    woT = nc.dram_tensor("woT", [INNER, C], F32R, kind="ExternalInput")
    bo2 = nc.dram_tensor("bo2", [2, 128, 1], F32, kind="ExternalInput")
    maskK = nc.dram_tensor("maskK", [NBLK + 1, GCOL], F32, kind="ExternalInput")
    maskQ = nc.dram_tensor("maskQ", [NBLK + 1, CHUNK], F32, kind="ExternalInput")
    yT = nc.dram_tensor("yT", [BPC, C, COLS], F32, kind="ExternalOutput")

    with tile.TileContext(nc) as tc:
        with (
            tc.tile_pool(name="const", bufs=1) as cst,
            tc.tile_pool(name="xin", bufs=sbuf_bufs + 1) as xin,
            tc.tile_pool(name="qk", bufs=sbuf_bufs) as qkp,
            tc.tile_pool(name="vsb", bufs=sbuf_bufs) as vsb,
            tc.tile_pool(name="esb", bufs=sbuf_bufs + 1) as esb,
            tc.tile_pool(name="rsb", bufs=sbuf_bufs) as rsb,
            tc.tile_pool(name="aosb", bufs=sbuf_bufs) as aosb,
            tc.tile_pool(name="ysb", bufs=sbuf_bufs) as ysb,
            tc.tile_pool(name="psm", bufs=psm_bufs, space="PSUM") as psm,
            tc.tile_pool(name="psS", bufs=pss_bufs, space="PSUM") as psS,
            tc.tile_pool(name="psD", bufs=psd_bufs, space="PSUM") as psD,
        ):
            # ---- constants ----
            wq_sb = [cst.tile([128, INNER], F32R, tag=f"wq{k}", name=f"wq{k}") for k in range(2)]
            wk_sb = [cst.tile([128, INNER], F32R, tag=f"wk{k}", name=f"wk{k}") for k in range(2)]
            wv_sb = [cst.tile([128, INNER], F32R, tag=f"wv{k}", name=f"wv{k}") for k in range(2)]
            wo_sb = [cst.tile([128, C], F32R, tag=f"wo{k}", name=f"wo{k}") for k in range(4)]
            for k in range(2):
                nc.sync.dma_start(wq_sb[k][:], wqT[128 * k : 128 * k + 128, :])
                nc.sync.dma_start(wk_sb[k][:], wkT[128 * k : 128 * k + 128, :])
                nc.sync.dma_start(wv_sb[k][:], wvT[128 * k : 128 * k + 128, :])
            for k in range(4):
                nc.sync.dma_start(wo_sb[k][:], woT[128 * k : 128 * k + 128, :])
            bo_t = [cst.tile([128, 1], F32, tag=f"bo{m}", name=f"bo{m}") for m in range(2)]
            for m in range(2):
                nc.sync.dma_start(bo_t[m][:], bo2[m, :, :])
            mk_f = cst.tile([NBLK + 1, GCOL], F32, tag="mkf", name="mkf")
            mq_f = cst.tile([NBLK + 1, CHUNK], F32, tag="mqf", name="mqf")
            nc.sync.dma_start(mk_f[:], maskK[:])
            nc.sync.dma_start(mq_f[:], maskQ[:])
            mk_b = cst.tile([NBLK + 1, GCOL], BF16, tag="mkb", name="mkb")
            mq_b = cst.tile([NBLK + 1, CHUNK], BF16, tag="mqb", name="mqb")
            nc.vector.tensor_copy(mk_b[:], mk_f[:])
            nc.vector.tensor_copy(mq_b[:], mq_f[:])
            ones_b = cst.tile([GCOL, 64], BF16, tag="ones", name="ones")
            nc.vector.memset(ones_b[:], 1.0)


            def copy_ps(dst, srcp, i):
                if evac == "alt":
                    eng = nc.scalar if i % 2 == 0 else nc.vector
                elif evac == "dve":
                    eng = nc.vector
                elif evac == "act":
                    eng = nc.scalar
                else:  # qk on dve, v on act
                    eng = nc.vector if i < 8 else nc.scalar
                if eng is nc.scalar:
                    nc.scalar.copy(dst, srcp)
                else:
                    nc.vector.tensor_copy(dst, srcp)

            # ---- main loop ----
            def _body():
              for b in range(BPC):
                for ch in range(NCHUNK):
                    c0 = CHUNK * ch
                    x_a = xin.tile([128, CHUNK], F32R, tag="xa", name="xa")
                    x_b = xin.tile([128, CHUNK], F32R, tag="xb", name="xb")
                    nc.sync.dma_start(x_a[:], xT[b, 0:128, c0 : c0 + CHUNK])
                    nc.sync.dma_start(x_b[:], xT[b, 128:256, c0 : c0 + CHUNK])

                    # Q^T and K^T projections: psum (128, CHUNK) x4 each
                    q_sb = [qkp.tile([128, CHUNK], BF16, tag=f"q{m}", name=f"q{m}") for m in range(4)]
                    k_sb = [qkp.tile([128, CHUNK], BF16, tag=f"k{m}", name=f"k{m}") for m in range(4)]
                    for m in (range(4) if "qk" not in ablate else []):
                        ms = slice(128 * m, 128 * m + 128)
                        pq = psm.tile([128, CHUNK], F32, tag="ps", name="ps")
                        nc.tensor.matmul(
                            pq[:], wq_sb[0][:, ms],
                            x_a[:], start=True, stop=False,
                        )
                        nc.tensor.matmul(
                            pq[:], wq_sb[1][:, ms],
                            x_b[:], start=False, stop=True,
                        )
                        copy_ps(q_sb[m][:], pq[:], 2 * m)
                        pk = psm.tile([128, CHUNK], F32, tag="ps", name="ps")
                        nc.tensor.matmul(
                            pk[:], wk_sb[0][:, ms],
                            x_a[:], start=True, stop=False,
                        )
                        nc.tensor.matmul(
                            pk[:], wk_sb[1][:, ms],
                            x_b[:], start=False, stop=True,
                        )
                        copy_ps(k_sb[m][:], pk[:], 2 * m + 1)

                    # V in row layout: (98 cols, 512) x5
                    v_sb = [vsb.tile([GCOL, INNER], BF16, tag=f"v{g}", name=f"v{g}") for g in range(NG)]
                    for g in (range(NG) if "v" not in ablate else []):
                        gs = slice(GCOL * g, GCOL * g + GCOL)
                        pv = psm.tile([GCOL, INNER], F32, tag="ps", name="ps")
                        nc.tensor.matmul(
                            pv[:], x_a[:, gs],
                            wv_sb[0][:], start=True, stop=False,
                        )
                        nc.tensor.matmul(
                            pv[:], x_b[:, gs],
                            wv_sb[1][:], start=False, stop=True,
                        )
                        copy_ps(v_sb[g][:], pv[:], 8 + g)

                    ao_sb = []
                    for half in range(2):
                        e_sb = []
                        for hh in range(4):
                            h = 4 * half + hh
                            ht, hp = h // 2, 64 * (h % 2)
                            if "s" not in ablate:
                                ps_s = psS.tile([GCOL, CHUNK], F32, tag="s", name="s")
                                nc.tensor.matmul(
                                    ps_s[:], mk_b[:], mq_b[:], start=True, stop=False,
                                )
                                for g in range(NG):
                                    gs = slice(GCOL * g, GCOL * g + GCOL)
                                    nc.tensor.matmul(
                                        ps_s[:, gs],
                                        k_sb[ht][hp : hp + 64, gs],
                                        q_sb[ht][hp : hp + 64, gs],
                                        start=False, stop=(g == NG - 1),
                                    )
                            eb = esb.tile([GCOL, CHUNK], BF16, tag=f"e{hh}", name=f"e{hh}")
                            if "exp" not in ablate and "s" not in ablate:
                                nc.scalar.activation(
                                    eb[:], ps_s[:], mybir.ActivationFunctionType.Exp
                                )
                            e_sb.append(eb)
                        for pr in range(2):
                            # bank-padded (512 free) so partition-base-64 slices stay bank-aligned
                            if "denom" not in ablate:
                                ps_d = psD.tile([128, 512], F32, tag="d", name="d")
                                nc.tensor.matmul(
                                    ps_d[0:64, 0:CHUNK], ones_b[:], e_sb[2 * pr][:],
                                    start=True, stop=True,
                                )
                                nc.tensor.matmul(
                                    ps_d[64:128, 0:CHUNK], ones_b[:], e_sb[2 * pr + 1][:],
                                    start=True, stop=True, tile_position=(0, 64),
                                )
                            if "av" not in ablate:
                                ps_av = psm.tile([128, 512], F32, tag="ps", name="ps")
                            for g in (range(NG) if "av" not in ablate else []):
                                gs = slice(GCOL * g, GCOL * g + GCOL)
                                for lo in range(2):
                                    h = 4 * half + 2 * pr + lo
                                    kwargs = {"tile_position": (0, 64)} if lo else {}
                                    nc.tensor.matmul(
                                        ps_av[64 * lo : 64 * lo + 64, gs],
                                        v_sb[g][:, 64 * h : 64 * h + 64],
                                        e_sb[2 * pr + lo][:, gs],
                                        start=(g == 0), stop=(g == 0),
                                        skip_group_check=(g > 0),
                                        **kwargs,
                                    )
                            aot = None
                            if ("norm" not in ablate) or ("y" not in ablate):
                                aot = aosb.tile([128, CHUNK], F32R, tag=f"ao{2 * half + pr}", name=f"ao{2 * half + pr}")
                            if "norm" not in ablate:
                                rc = rsb.tile([128, CHUNK], F32, tag="rc", name="rc")
                                nc.vector.reciprocal(rc[:], ps_d[:, 0:CHUNK])
                                nc.vector.tensor_mul(aot[:], ps_av[:, 0:CHUNK], rc[:])
                            ao_sb.append(aot)

                    # output projection Y = Wo @ AO + bo
                    for mo in (range(2) if "y" not in ablate else []):
                        mos = slice(128 * mo, 128 * mo + 128)
                        py = psm.tile([128, CHUNK], F32, tag="ps", name="ps")
                        for k in range(4):
                            nc.tensor.matmul(
                                py[:], wo_sb[k][:, mos],
                                ao_sb[k][:],
                                start=(k == 0), stop=(k == 3),
                            )
                        yo = ysb.tile([128, CHUNK], F32, tag=f"y{mo}", name=f"y{mo}")
                        nc.scalar.activation(
                            yo[:], py[:],
                            mybir.ActivationFunctionType.Identity,
                            bias=bo_t[mo][:],
                        )
                        nc.sync.dma_start(yT[b, mos, c0 : c0 + CHUNK], yo[:])

            if hwloop and repeat > 1:
                with tc.For_i(0, repeat):
                    _body()
            else:
                for _rep in range(repeat):
                    _body()

    nc.compile()
    return nc


def _host_inputs(fmap, Wq, Wkv, Wo, bo):
    t = T // P
    # (b, c, T, V) -> (b, c, t, p, v) -> (b, c, t, v, p) -> (b, c, cols)
    xT = np.ascontiguousarray(
        fmap.reshape(B, C, t, P, V).transpose(0, 1, 2, 4, 3).reshape(B, C, COLS)
    ).astype(np.float32)
    wqT = np.ascontiguousarray(Wq.T * np.float32(DH**-0.5)).astype(np.float32)
    wkT = np.ascontiguousarray(Wkv[:INNER].T).astype(np.float32)
    wvT = np.ascontiguousarray(Wkv[INNER:].T).astype(np.float32)
    woT = np.ascontiguousarray(Wo.T).astype(np.float32)
    bo2 = bo.reshape(2, 128, 1).astype(np.float32)

    mk = np.zeros((NBLK + 1, GCOL), np.float32)
    mq = np.zeros((NBLK + 1, CHUNK), np.float32)
    mk[0, :] = 1.0
    mq[0, :] = -MASKC
    for g in range(NBLK):
        mk[1 + g, P * g : P * g + P] = 1.0
        for rep in range(NG):
            mq[1 + g, GCOL * rep + P * g : GCOL * rep + P * g + P] = MASKC
    return xT, dict(wqT=wqT, wkT=wkT, wvT=wvT, woT=woT, bo2=bo2, maskK=mk, maskQ=mq)


def _unpermute(y):  # (B, C, COLS) -> (B, C, T, V)
    t = T // P
    return np.ascontiguousarray(
        y.reshape(B, C, t, V, P).transpose(0, 1, 2, 4, 3).reshape(B, C, T, V)
    ).astype(np.float32)


def kernel(fmap, Wq, Wkv, Wo, bo):
    if "nc" not in _CACHE:
        _CACHE["nc"] = _build_program()
    nc = _CACHE["nc"]
    xT, shared = _host_inputs(
        np.asarray(fmap), np.asarray(Wq), np.asarray(Wkv), np.asarray(Wo), np.asarray(bo)
    )
    in_maps = [
        {"xT": np.ascontiguousarray(xT[BPC * c : BPC * c + BPC]), **shared}
        for c in range(NCORES)
    ]
    res = bass_utils.run_bass_kernel_spmd(nc, in_maps, core_ids=list(range(NCORES)))
    y = np.concatenate([res.results[c]["yT"] for c in range(NCORES)], axis=0)
    return _unpermute(y)


if __name__ == "__main__":
    # quick self-run with random data
    rng = np.random.default_rng(0)
    fmap = rng.standard_normal((B, C, T, V), dtype=np.float32)
    Wq = (rng.standard_normal((INNER, C)) * 0.02).astype(np.float32)
    Wkv = (rng.standard_normal((2 * INNER, C)) * 0.02).astype(np.float32)
    Wo = (rng.standard_normal((C, INNER)) * 0.02).astype(np.float32)
    bo = np.zeros((C,), np.float32)
    y = kernel(fmap=fmap, Wq=Wq, Wkv=Wkv, Wo=Wo, bo=bo)
    print("out", y.shape, y.dtype, float(np.abs(y).mean()))



# revision 40
# speedup vs baseline: 1.1643x; 1.1643x over previous
"""Trainium2 Bass kernel for nn_LocalAttention (block-local attention, P=7).

Strategy
--------
Data-parallel over batch b: 16 batches -> 8 cores x 2 batches.

Host side: fmap is permuted to X^T layout (b, c, cols) with cols ordered
(t, v, p) so each attention block (t,v) is 7 contiguous columns. Weights are
pre-transposed (contraction dim c / inner on partitions); the attention scale
DH**-0.5 = 0.125 is folded into Wq. The softmax mask (+32 on block-diagonal,
-32 uniform) is passed as two rank-15 factor matrices so the PE adds it into
the score PSUM with one extra accumulating matmul; exp() then yields ~1e-14
off block-diagonal, i.e. exact-enough masking with zero vector-engine work.

On chip per 490-column chunk (5 groups of 14 blocks = 98 cols):
  Q^T/K^T (inner x cols) and V (cols x inner) projections in fp32r,
  S^T = K^T.T @ Q^T per (group, head) in bf16 accumulated onto the mask
  matmul, one batched Exp per PSUM bank (ScalarE, psum->sbuf bf16),
  denominators via ones-matmul broadcast to 64 partitions (head-pair packed),
  AV matmuls head-pair packed, VectorE reciprocal + tensor_mul (= fused
  softmax-normalize + PSUM evacuation), fp32r output projection + bias,
  DMA out.  PSUM evacuations rotate over Pool/Activation/Vector engines;
  startup weight DMAs are split across both HWDGE queues (SP + Activation)
  so the first projections start early.  Output is un-permuted on host.
"""

import os
import sys

for _p in ("/opt/trn_rl_repo", "/root/.axon_site/_ro/trn_rl_repo"):
    if os.path.isdir(_p) and _p not in sys.path:
        sys.path.insert(0, _p)

import numpy as np

import concourse.bacc as bacc
import concourse.mybir as mybir
import concourse.tile as tile
from concourse import bass_utils

F32 = mybir.dt.float32
F32R = mybir.dt.float32r
BF16 = mybir.dt.bfloat16

# problem constants
B, C, T, V = 16, 256, 196, 25
P, H, DH = 7, 8, 64
INNER = H * DH  # 512
NCORES = 8
BPC = B // NCORES  # batches per core = 2
COLS = T * V  # 4900 columns per batch (t, v, p)-ordered
CHUNK = 490  # columns per processing chunk
NCHUNK = COLS // CHUNK  # 10
GCOL = 98  # columns per attention group (14 blocks of 7)
NG = CHUNK // GCOL  # 5 groups per chunk
NBLK = GCOL // P  # 14 blocks per group
MASKR = NBLK + 1  # mask rank 15
MASKC = 32.0  # block-diagonal boost; exp bias is -MASKC via uniform rank-1 term

_CACHE: dict = {}


def _build_program(
    repeat: int = 1,
    ablate: frozenset = frozenset(),
    psm_bufs: int = 4,
    pss_bufs: int = 3,
    psd_bufs: int = 1,
    evac: str = "alt",
    sbuf_bufs: int = 2,
    xin_bufs: int = 3,
    hwloop: bool = True,
    defer_y: bool = True,
    y_dma_eng: str = "sp",
    ybias_eng: str = "act",
    swdge_evac: int = 0,  # evacs per chunk offloaded to Pool-issued SWDGE DMA
):
    nc = bacc.Bacc("TRN2", target_bir_lowering=False, debug=False)

    xT = nc.dram_tensor("xT", [BPC, C, COLS], F32R, kind="ExternalInput")
    wqT = nc.dram_tensor("wqT", [C, INNER], F32R, kind="ExternalInput")
    wkT = nc.dram_tensor("wkT", [C, INNER], F32R, kind="ExternalInput")
    wvT = nc.dram_tensor("wvT", [C, INNER], F32R, kind="ExternalInput")
    woT = nc.dram_tensor("woT", [INNER, C], F32R, kind="ExternalInput")
    bo2 = nc.dram_tensor("bo2", [2, 128, 1], F32, kind="ExternalInput")
    maskKB = nc.dram_tensor("maskKB", [MASKR, GCOL], BF16, kind="ExternalInput")
    maskQB = nc.dram_tensor("maskQB", [MASKR, CHUNK], BF16, kind="ExternalInput")
    yT = nc.dram_tensor("yT", [BPC, C, COLS], F32, kind="ExternalOutput")

    with tile.TileContext(nc) as tc:
        with (
            tc.tile_pool(name="const", bufs=1) as cst,
            tc.tile_pool(name="xin", bufs=xin_bufs) as xin,
            tc.tile_pool(name="qk", bufs=sbuf_bufs) as qkp,
            tc.tile_pool(name="vsb", bufs=sbuf_bufs) as vsb,
            tc.tile_pool(name="esb", bufs=sbuf_bufs + 1) as esb,
            tc.tile_pool(name="rsb", bufs=sbuf_bufs) as rsb,
            tc.tile_pool(name="aosb", bufs=sbuf_bufs) as aosb,
            tc.tile_pool(name="ysb", bufs=sbuf_bufs) as ysb,
            tc.tile_pool(name="psm", bufs=psm_bufs, space="PSUM") as psm,
            tc.tile_pool(name="psS", bufs=pss_bufs, space="PSUM") as psS,
            tc.tile_pool(name="psD", bufs=psd_bufs, space="PSUM") as psD,
        ):
            # ---- constants: weights split across both HWDGE queues ----
            wq_sb = [cst.tile([128, INNER], F32R, tag=f"wq{k}", name=f"wq{k}") for k in range(2)]
            wk_sb = [cst.tile([128, INNER], F32R, tag=f"wk{k}", name=f"wk{k}") for k in range(2)]
            wv_sb = [cst.tile([128, INNER], F32R, tag=f"wv{k}", name=f"wv{k}") for k in range(2)]
            wo_sb = [cst.tile([128, C], F32R, tag=f"wo{k}", name=f"wo{k}") for k in range(4)]
            # all constants go on the Activation HWDGE queue, need-first order,
            # leaving the SP queue free for the first x tiles
            bo_t = [cst.tile([128, 1], F32, tag=f"bo{m}", name=f"bo{m}") for m in range(2)]
            mk_b = cst.tile([MASKR, GCOL], BF16, tag="mkb", name="mkb")
            mq_b = cst.tile([MASKR, CHUNK], BF16, tag="mqb", name="mqb")
            for k in range(2):
                nc.scalar.dma_start(wq_sb[k][:], wqT[128 * k : 128 * k + 128, :])
            for k in range(2):
                nc.scalar.dma_start(wk_sb[k][:], wkT[128 * k : 128 * k + 128, :])
            for k in range(2):
                nc.scalar.dma_start(wv_sb[k][:], wvT[128 * k : 128 * k + 128, :])
            nc.scalar.dma_start(mk_b[:], maskKB[:])
            nc.scalar.dma_start(mq_b[:], maskQB[:])
            for k in range(4):
                nc.scalar.dma_start(wo_sb[k][:], woT[128 * k : 128 * k + 128, :])
            for m in range(2):
                nc.scalar.dma_start(bo_t[m][:], bo2[m, :, :])
            ones_b = cst.tile([GCOL, 64], BF16, tag="ones", name="ones")
            nc.vector.memset(ones_b[:], 1.0)

            evac_state = {"i": 0}

            def copy_ps(dst, srcp):
                i = evac_state["i"]
                evac_state["i"] += 1
                if swdge_evac and (i % 13) < swdge_evac:
                    # Pool-issued software-DGE DMA: casts f32 psum -> bf16 sbuf
                    # on the DMA engines, costing the Pool engine only the
                    # descriptor trigger.
                    nc.gpsimd.dma_start(dst, srcp)
                    return
                if evac == "alt":
                    eng = nc.scalar if i % 2 == 0 else nc.vector
                elif evac == "alt_d":
                    eng = nc.vector if i % 2 == 0 else nc.scalar
                elif evac == "dve":
                    eng = nc.vector
                else:
                    eng = nc.scalar
                if eng is nc.scalar:
                    nc.scalar.copy(dst, srcp)
                else:
                    eng.tensor_copy(dst, srcp)

            def emit_x(b, ch):
                c0 = CHUNK * ch
                x_a = xin.tile([128, CHUNK], F32R, tag="xa", name="xa")
                x_b = xin.tile([128, CHUNK], F32R, tag="xb", name="xb")
                nc.sync.dma_start(x_a[:], xT[b, 0:128, c0 : c0 + CHUNK])
                nc.sync.dma_start(x_b[:], xT[b, 128:256, c0 : c0 + CHUNK])
                return x_a, x_b

            def emit_qkv(x_a, x_b):
                # Q^T and K^T projections: psum (128, CHUNK) x4 each
                q_sb = [qkp.tile([128, CHUNK], BF16, tag=f"q{m}", name=f"q{m}") for m in range(4)]
                k_sb = [qkp.tile([128, CHUNK], BF16, tag=f"k{m}", name=f"k{m}") for m in range(4)]
                for m in (range(4) if "qk" not in ablate else []):
                    ms = slice(128 * m, 128 * m + 128)
                    pq = psm.tile([128, CHUNK], F32, tag="ps", name="ps")
                    nc.tensor.matmul(pq[:], wq_sb[0][:, ms], x_a[:], start=True, stop=False)
                    nc.tensor.matmul(pq[:], wq_sb[1][:, ms], x_b[:], start=False, stop=True)
                    copy_ps(q_sb[m][:], pq[:])
                    pk = psm.tile([128, CHUNK], F32, tag="ps", name="ps")
                    nc.tensor.matmul(pk[:], wk_sb[0][:, ms], x_a[:], start=True, stop=False)
                    nc.tensor.matmul(pk[:], wk_sb[1][:, ms], x_b[:], start=False, stop=True)
                    copy_ps(k_sb[m][:], pk[:])

                # V in row layout: (98 cols, 512) x5
                v_sb = [vsb.tile([GCOL, INNER], BF16, tag=f"v{g}", name=f"v{g}") for g in range(NG)]
                for g in (range(NG) if "v" not in ablate else []):
                    gs = slice(GCOL * g, GCOL * g + GCOL)
                    pv = psm.tile([GCOL, INNER], F32, tag="ps", name="ps")
                    nc.tensor.matmul(pv[:], x_a[:, gs], wv_sb[0][:], start=True, stop=False)
                    nc.tensor.matmul(pv[:], x_b[:, gs], wv_sb[1][:], start=False, stop=True)
                    copy_ps(v_sb[g][:], pv[:])
                return q_sb, k_sb, v_sb

            def emit_sdav(q_sb, k_sb, v_sb):
                ao_sb = []
                for half in range(2):
                    e_sb = []
                    for hh in range(4):
                        h = 4 * half + hh
                        ht, hp = h // 2, 64 * (h % 2)
                        if "s" not in ablate:
                            ps_s = psS.tile([GCOL, CHUNK], F32, tag="s", name="s")
                            nc.tensor.matmul(
                                ps_s[:], mk_b[:], mq_b[:], start=True, stop=False,
                            )
                            for g in range(NG):
                                gs = slice(GCOL * g, GCOL * g + GCOL)
                                nc.tensor.matmul(
                                    ps_s[:, gs],
                                    k_sb[ht][hp : hp + 64, gs],
                                    q_sb[ht][hp : hp + 64, gs],
                                    start=False, stop=(g == NG - 1),
                                )
                        eb = esb.tile([GCOL, CHUNK], BF16, tag=f"e{hh}", name=f"e{hh}")
                        if "exp" not in ablate and "s" not in ablate:
                            nc.scalar.activation(
                                eb[:], ps_s[:], mybir.ActivationFunctionType.Exp
                            )
                        e_sb.append(eb)
                    for pr in range(2):
                        # bank-padded (512 free) so partition-base-64 slices stay bank-aligned
                        if "denom" not in ablate:
                            ps_d = psD.tile([128, 512], F32, tag="d", name="d")
                            nc.tensor.matmul(
                                ps_d[0:64, 0:CHUNK], ones_b[:], e_sb[2 * pr][:],
                                start=True, stop=True,
                            )
                            nc.tensor.matmul(
                                ps_d[64:128, 0:CHUNK], ones_b[:], e_sb[2 * pr + 1][:],
                                start=True, stop=True, tile_position=(0, 64),
                            )
                        if "av" not in ablate:
                            ps_av = psm.tile([128, 512], F32, tag="ps", name="ps")
                        for g in (range(NG) if "av" not in ablate else []):
                            gs = slice(GCOL * g, GCOL * g + GCOL)
                            for lo in range(2):
                                h = 4 * half + 2 * pr + lo
                                kwargs = {"tile_position": (0, 64)} if lo else {}
                                nc.tensor.matmul(
                                    ps_av[64 * lo : 64 * lo + 64, gs],
                                    v_sb[g][:, 64 * h : 64 * h + 64],
                                    e_sb[2 * pr + lo][:, gs],
                                    start=(g == 0), stop=(g == 0),
                                    skip_group_check=(g > 0),
                                    **kwargs,
                                )
                        aot = None
                        if ("norm" not in ablate) or ("y" not in ablate):
                            aot = aosb.tile([128, CHUNK], F32R, tag=f"ao{2 * half + pr}", name=f"ao{2 * half + pr}")
                        if "norm" not in ablate:
                            rc = rsb.tile([128, CHUNK], F32, tag="rc", name="rc")
                            nc.vector.reciprocal(rc[:], ps_d[:, 0:CHUNK])
                            nc.vector.tensor_mul(aot[:], ps_av[:, 0:CHUNK], rc[:])
                        ao_sb.append(aot)
                return ao_sb

            def emit_y(ao_sb, b, ch):
                c0 = CHUNK * ch
                for mo in (range(2) if "y" not in ablate else []):
                    mos = slice(128 * mo, 128 * mo + 128)
                    py = psm.tile([128, CHUNK], F32, tag="ps", name="ps")
                    for k in range(4):
                        nc.tensor.matmul(
                            py[:], wo_sb[k][:, mos],
                            ao_sb[k][:],
                            start=(k == 0), stop=(k == 3),
                        )
                    yo = ysb.tile([128, CHUNK], F32, tag=f"y{mo}", name=f"y{mo}")
                    if ybias_eng == "act":
                        nc.scalar.activation(
                            yo[:], py[:],
                            mybir.ActivationFunctionType.Identity,
                            bias=bo_t[mo][:],
                        )
                    else:
                        eng = nc.gpsimd if ybias_eng == "pool" else nc.vector
                        eng.tensor_scalar(
                            yo[:], py[:], bo_t[mo][:],
                            scalar2=None, op0=mybir.AluOpType.add,
                        )
                    (nc.scalar if y_dma_eng == "act" else nc.sync).dma_start(
                        yT[b, mos, c0 : c0 + CHUNK], yo[:]
                    )

            steps = [(b, ch) for b in range(BPC) for ch in range(NCHUNK)]

            def _body():
                xt = emit_x(*steps[0])
                if defer_y:
                    pend_y = None
                    last = len(steps) - 1
                    for i, (b, ch) in enumerate(steps):
                        q_sb, k_sb, v_sb = emit_qkv(*xt)
                        if i < last:
                            xt = emit_x(*steps[i + 1])
                        if pend_y is not None:
                            emit_y(*pend_y)
                        ao = emit_sdav(q_sb, k_sb, v_sb)
                        if i == last:
                            emit_y(ao, b, ch)
                        else:
                            pend_y = (ao, b, ch)
                else:
                    for i, (b, ch) in enumerate(steps):
                        q_sb, k_sb, v_sb = emit_qkv(*xt)
                        if i + 1 < len(steps):
                            xt = emit_x(*steps[i + 1])
                        ao = emit_sdav(q_sb, k_sb, v_sb)
                        emit_y(ao, b, ch)

            if hwloop and repeat > 1:
                with tc.For_i(0, repeat):
                    _body()
            else:
                for _rep in range(repeat):
                    _body()

    nc.compile()
    return nc


def _host_inputs(fmap, Wq, Wkv, Wo, bo):
    import ml_dtypes

    t = T // P
    # (b, c, T, V) -> (b, c, t, p, v) -> (b, c, t, v, p) -> (b, c, cols)
    xT = np.ascontiguousarray(
        fmap.reshape(B, C, t, P, V).transpose(0, 1, 2, 4, 3).reshape(B, C, COLS)
    ).astype(np.float32)
    wqT = np.ascontiguousarray(Wq.T * np.float32(DH**-0.5)).astype(np.float32)
    wkT = np.ascontiguousarray(Wkv[:INNER].T).astype(np.float32)
    wvT = np.ascontiguousarray(Wkv[INNER:].T).astype(np.float32)
    woT = np.ascontiguousarray(Wo.T).astype(np.float32)
    bo2 = bo.reshape(2, 128, 1).astype(np.float32)

    mk = np.zeros((MASKR, GCOL), np.float32)
    mq = np.zeros((MASKR, CHUNK), np.float32)
    mk[0, :] = 1.0
    mq[0, :] = -MASKC
    for g in range(NBLK):
        mk[1 + g, P * g : P * g + P] = 1.0
        for rep in range(NG):
            mq[1 + g, GCOL * rep + P * g : GCOL * rep + P * g + P] = MASKC
    maskKB = mk.astype(ml_dtypes.bfloat16)
    maskQB = mq.astype(ml_dtypes.bfloat16)
    return xT, dict(
        wqT=wqT, wkT=wkT, wvT=wvT, woT=woT, bo2=bo2, maskKB=maskKB, maskQB=maskQB
    )


def _unpermute(y):  # (B, C, COLS) -> (B, C, T, V)
    t = T // P
    return np.ascontiguousarray(
        y.reshape(B, C, t, V, P).transpose(0, 1, 2, 4, 3).reshape(B, C, T, V)
    ).astype(np.float32)


def kernel(fmap, Wq, Wkv, Wo, bo):
    if "nc" not in _CACHE:
        _CACHE["nc"] = _build_program()
    nc = _CACHE["nc"]
    xT, shared = _host_inputs(
        np.asarray(fmap), np.asarray(Wq), np.asarray(Wkv), np.asarray(Wo), np.asarray(bo)
    )
    in_maps = [
        {"xT": np.ascontiguousarray(xT[BPC * c : BPC * c + BPC]), **shared}
        for c in range(NCORES)
    ]
    res = bass_utils.run_bass_kernel_spmd(nc, in_maps, core_ids=list(range(NCORES)))
    y = np.concatenate([res.results[c]["yT"] for c in range(NCORES)], axis=0)
    return _unpermute(y)


if __name__ == "__main__":
    # quick self-run with random data
    rng = np.random.default_rng(0)
    fmap = rng.standard_normal((B, C, T, V), dtype=np.float32)
    Wq = (rng.standard_normal((INNER, C)) * 0.02).astype(np.float32)
    Wkv = (rng.standard_normal((2 * INNER, C)) * 0.02).astype(np.float32)
    Wo = (rng.standard_normal((C, INNER)) * 0.02).astype(np.float32)
    bo = np.zeros((C,), np.float32)
    y = kernel(fmap=fmap, Wq=Wq, Wkv=Wkv, Wo=Wo, bo=bo)
    print("out", y.shape, y.dtype, float(np.abs(y).mean()))
